# revision 4
# baseline (speedup 1.0000x reference)
"""Trainium2 Bass kernel for CompactPiecewiseLinearEmbeddings.

out[n, f*8+d] = sum_b h[n,f,b] * W[f,b,d] + b[f,d]
h = piecewise-linear encoding of x[n,f] over per-feature bins
    (first bin clamp_max(1), middle clamp(0,1), last bin clamp_min(0)).

Strategy (per core; data-parallel over N across 8 cores):
 - Host pre-transposes x into a padded [512, NS] layout: 16 groups of 32
   partition rows, each [16 features, ones-row, 15 pad].
 - PE float32r matmul computes the per-bin affine directly:
   s[f,j] = winv[f,j]*x[f] - e[f,j]*winv[f,j]  (ones-row carries the bias),
   packed flat 128 bin-rows per tile, 6 tiles per 16-feature group.
 - ONE elementwise pass clamps s -> h (bf16): either a DVE dual-op
   tensor_scalar (max(maxv), min(minv)) straight from PSUM, or an
   ACT Identity evacuation followed by the same dual-op on GPSIMD.
   Edge bins use +-440 bounds (bin0: [-440, 1], bin47: [0, 440]).
 - PE bf16 matmul contracts h (128 rows) against the block-diagonal W
   into [128 fd, n] PSUM tiles (6 accumulating matmuls per group).
 - ACT evacuates + adds output bias -> bf16 SBUF -> DMA out in [fd, n]
   layout. Host transposes/casts to the final [n, fd] f32.
"""
import numpy as np
import ml_dtypes

from concourse import bacc, mybir
from concourse.tile import TileContext
from concourse.bass_utils import run_bass_kernel_spmd

N, F, B, D = 16384, 256, 48, 8
NCORES = 8
NS = N // NCORES          # 2048 rows per core
CH = 512                  # n-chunk (matmul free dim / PSUM bank)
NCH = NS // CH            # 4
NG = 16                   # feature groups (16 features each)
TPG = 6                   # h-tiles per group (16*48/128)
NT = NG * TPG             # 96 h-tiles
EDGE_CAP = 1e30           # edge bins are unbounded on one side (bf16-safe)

# route B fraction: ACT copy + GPSIMD clamp (vs DVE direct clamp)
ROUTE_B_MOD = 15
ROUTE_B_CNT = 7

_cache = {}


def build_nc():
    nc = bacc.Bacc("TRN2")
    f32, bf16 = mybir.dt.float32, mybir.dt.bfloat16
    f32r = mybir.dt.float32r

    xT_ext = nc.declare_dram_parameter("xT", [4 * 128, NS], f32r, isOutput=False)
    sel_ext = nc.declare_dram_parameter("selAff", [128, 4 * TPG * 128], f32r, isOutput=False)
    wpk_ext = nc.declare_dram_parameter("wpack", [128, NT * 128], bf16, isOutput=False)
    obias_ext = nc.declare_dram_parameter("obias", [128, NG], f32, isOutput=False)
    maxv_ext = nc.declare_dram_parameter("maxv", [128, NT], f32, isOutput=False)
    minv_ext = nc.declare_dram_parameter("minv", [128, NT], f32, isOutput=False)
    out_ext = nc.declare_dram_parameter("out", [F * D, NS], bf16, isOutput=True)

    Ident = mybir.ActivationFunctionType.Identity
    amax, amin = mybir.AluOpType.max, mybir.AluOpType.min

    with TileContext(nc) as tc:
        with (
            tc.tile_pool(name="const", bufs=1) as cpool,
            tc.tile_pool(name="hbuf", bufs=8) as hpool,
            tc.tile_pool(name="sbuf2", bufs=6) as spool,
            tc.tile_pool(name="osb", bufs=6) as opool,
            tc.tile_pool(name="bc", bufs=3, space="PSUM") as bcpool,
            tc.tile_pool(name="oc", bufs=5, space="PSUM") as ocpool,
        ):
            # ---- constants ----
            xT = [cpool.tile([128, NS], f32r, tag=f"xT{i}", name=f"xT{i}")
                  for i in range(4)]
            for i in range(4):
                nc.sync.dma_start(out=xT[i][:], in_=xT_ext[i * 128:(i + 1) * 128, :])
            sel = cpool.tile([128, 4 * TPG * 128], f32r)
            wpk = cpool.tile([128, NT * 128], bf16)
            obias = cpool.tile([128, NG], f32)
            maxv = cpool.tile([128, NT], f32)
            minv = cpool.tile([128, NT], f32)
            for t, e in [(sel, sel_ext), (wpk, wpk_ext), (obias, obias_ext),
                         (maxv, maxv_ext), (minv, minv_ext)]:
                nc.sync.dma_start(out=t[:], in_=e[:])

            # ---- main loop ----
            idx = 0
            for c in range(NCH):
                for g in range(NG):
                    q = g % 4
                    xt = xT[g // 4]
                    oc = ocpool.tile([128, CH], f32, tag="oc")
                    for t in range(TPG):
                        gi = g * TPG + t
                        ps = bcpool.tile([128, CH], f32, tag="ps")
                        selcol = ((g // 4) * TPG + t) * 128
                        nc.tensor.matmul(
                            ps[:],
                            sel[32 * q:32 * q + 32, selcol:selcol + 128],
                            xt[32 * q:32 * q + 32, c * CH:(c + 1) * CH],
                            start=True, stop=True,
                            tile_position=(32 * q, 0),
                        )
                        h = hpool.tile([128, CH], bf16, tag="h")
                        route_b = (idx % ROUTE_B_MOD) < ROUTE_B_CNT
                        idx += 1
                        if route_b:
                            s = spool.tile([128, CH], bf16, tag="s")
                            nc.scalar.activation(s[:], ps[:], Ident)
                            nc.gpsimd.tensor_scalar(
                                h[:], s[:], maxv[:, gi:gi + 1], minv[:, gi:gi + 1],
                                amax, amin,
                            )
                        else:
                            nc.vector.tensor_scalar(
                                h[:], ps[:], maxv[:, gi:gi + 1], minv[:, gi:gi + 1],
                                amax, amin,
                            )
                        nc.tensor.matmul(
                            oc[:],
                            wpk[:, gi * 128:(gi + 1) * 128],
                            h[:],
                            start=(t == 0), stop=(t == TPG - 1),
                        )
                    osb = opool.tile([128, CH], bf16, tag="osb")
                    nc.scalar.activation(osb[:], oc[:], Ident,
                                         bias=obias[:, g:g + 1])
                    nc.sync.dma_start(
                        out=out_ext[g * 128:(g + 1) * 128, c * CH:(c + 1) * CH],
                        in_=osb[:])

    nc.compile()
    return nc


def host_constants(edges, width, W, b):
    """Build packed constant tensors. edges/width [F,B], W [F,B,D], b [F,D]."""
    f32 = np.float32
    edges = np.asarray(edges, f32)
    width = np.asarray(width, f32)
    W = np.asarray(W, f32)
    b = np.asarray(b, f32)
    winv = (1.0 / width).astype(f32)

    # selAff: [128 partitions, 4*TPG*128]; band q=rows 32q..32q+31 serves
    # groups with g%4==q; within band: [16 feats, ones@16, 15 pad].
    sel = np.zeros((128, 4 * TPG * 128), f32)
    wpack = np.zeros((128, NT * 128), f32)
    obias = np.zeros((128, NG), f32)
    maxv = np.zeros((128, NT), f32)
    minv = np.zeros((128, NT), f32)

    for g in range(NG):
        q, blk = g % 4, g // 4
        for t in range(TPG):
            gi = g * TPG + t
            selcol = (blk * TPG + t) * 128
            for m in range(128):
                r = 128 * t + m          # row within the group (0..767)
                fl, j = r // B, r % B    # local feature, bin
                f = 16 * g + fl
                sel[32 * q + fl, selcol + m] = winv[f, j]
                sel[32 * q + 16, selcol + m] = -edges[f, j] * winv[f, j]
                if j == 0:
                    maxv[m, gi] = -EDGE_CAP
                    minv[m, gi] = 1.0
                elif j == B - 1:
                    maxv[m, gi] = 0.0
                    minv[m, gi] = EDGE_CAP
                else:
                    maxv[m, gi] = 0.0
                    minv[m, gi] = 1.0
                wpack[m, gi * 128 + 8 * fl:gi * 128 + 8 * fl + 8] = W[f, j, :]
        for fl in range(16):
            obias[8 * fl:8 * fl + 8, g] = b[16 * g + fl, :]

    return {
        "selAff": sel,
        "wpack": wpack.astype(ml_dtypes.bfloat16),
        "obias": obias,
        "maxv": maxv,
        "minv": minv,
    }


def make_xT(x_core):
    """x_core [NS, F] f32 -> padded transposed [512, NS] f32."""
    xT = np.zeros((4 * 128, NS), np.float32)
    xt_full = np.ascontiguousarray(x_core.T)          # [F, NS]
    for g in range(NG):
        base = 32 * (g % 4) + 128 * (g // 4)
        xT[base:base + 16, :] = xt_full[16 * g:16 * g + 16, :]
        xT[base + 16, :] = 1.0
    return xT


def make_in_maps(x, edges, width, W, b):
    consts = host_constants(edges, width, W, b)
    x = np.ascontiguousarray(np.asarray(x, dtype=np.float32))
    in_maps = []
    for core in range(NCORES):
        m = dict(consts)
        m["xT"] = make_xT(x[core * NS:(core + 1) * NS, :])
        in_maps.append(m)
    return in_maps


def kernel(x, edges, width, W, b):
    if "nc" not in _cache:
        _cache["nc"] = build_nc()
    nc = _cache["nc"]
    in_maps = make_in_maps(x, edges, width, W, b)
    res = run_bass_kernel_spmd(nc, in_maps, core_ids=list(range(NCORES)))
    outs = []
    for r in res.results:
        o = np.asarray(r["out"])                      # [F*D, NS] bf16
        outs.append(o.astype(np.float32).T)           # [NS, F*D]
    return np.ascontiguousarray(np.concatenate(outs, axis=0))
